# revision 21
# baseline (speedup 1.0000x reference)
"""Trainium2 Bass kernel for nn_CausalLayer (bilinear causal mixing layer).

Math (per batch b):
    E = ae[x]                                # [L, D] gather
    S[i,j] = E_i @ w @ E_j                   # bilinear pairwise score
    coef[i,j] = (i+1)/(j+1) for i<j else 0
    res[:,j] = bx[:,j] + sum_i coef[i,j]*S[i,j]*bx[:,i]

Chunked linear-attention identity (chunk C=128). With a_i = w^T E_i,
e'_j = E_j/(j+1) and y_i = (i+1)*bx_i:

    acc_j = M_cj @ e'_j + sum_{i<j, same chunk} ((i+1) a_i . e'_j) bx_i
    M_c   = sum_{i in chunks < c} y_i a_i^T      (rank-D running state, [D, H])
    res_j = bx_j + acc_j                          (final add on host)

Host prep (all O(L*D)): the ae gather, A = E @ w, the (i+1)/(1/(j+1)) row
and column scalings, and the [D, C] transposes.

Schedule notes (v9). The PE co-executes matmul pairs whose partition use is
disjoint: same-contract matmuls writing opposite output halves (col groups)
and 64-contract matmuls reading opposite input halves (row groups) overlap
fully, so wall time is the max of the pair, not the sum. The H=768 state is
therefore folded exactly in half: M lives as one [128, 384] PSUM tile
(partitions 0:64 = M[:, 0:384], 64:128 = M[:, 384:768]) so the M-update is
a col-group pair (384-col wall) and the state apply a row-group pair
(384-col wall). The per-chunk serialized PE wall is 768 (out1) + 128
(score) + 384 + 384 = 1664 columns. The critical cross-engine chain is
M-update(t) -> bf16 M snapshot -> M-update(t+1); the snapshot is one
[128, 384] Act copy so the chain fits inside the chunk period. The
PSUM->bf16 output copy is split DVE (cols 0:384, own SBUF tile) / Act
(cols 384:768, own tile) — separate tiles so the tracker never serializes
the two engines, and each group stores as two DMAs. Input posts are spread
across the Sync (etat/ap), GpSimd and Act queues (a single DMA queue
sustains only ~120 GB/s), with the first two groups' bx posted per-chunk
so each slice lands just before the pipeline reaches it.

Sharding: batch-parallel, 2 of 16 batches per NeuronCore across 8 cores;
all tables are per-core slices. No cross-core communication.
"""

import os
import sys

for _p in ("/opt/trn_rl_repo", "/root/.axon_site/_ro/trn_rl_repo"):
    if os.path.isdir(_p) and _p not in sys.path:
        sys.path.insert(0, _p)

import numpy as np

B, L, H = 16, 2048, 768
V, D = 30000, 64
NCORES = 8
BPC = B // NCORES          # batches per core
C = 128                    # chunk (tile) size along sequence
NCH = L // C               # chunks per batch
ROWS = BPC * L             # bx rows per core
NT = BPC * NCH             # total chunks per core
G = 4                      # chunks per DMA group
HF = H // 2                # exact fold: cols 0:384 on partitions 0:64, rest above

_compiled = {}


def _build():
    """Build + compile the per-core Bass module (SPMD: same program, 8 cores)."""
    key = "v10"
    if key in _compiled:
        return _compiled[key]

    import concourse.bacc as bacc
    import concourse.bass as bass
    import concourse.mybir as mybir
    import concourse.tile as tile

    f32 = mybir.dt.float32
    bf16 = mybir.dt.bfloat16

    nc = bacc.Bacc(
        "TRN2",
        target_bir_lowering=False,
        debug=False,
        enable_asserts=False,
        num_devices=NCORES,
    )

    bx_d = nc.dram_tensor("bx", [ROWS, H], bf16, kind="ExternalInput").ap()
    # per global chunk g: cols [g*2C, g*2C+C) = Et'_g ([D, C] transposed E,
    # column j scaled by 1/(j+1)), cols [g*2C+C, (g+1)*2C) = At'_g
    # ((i+1)-scaled transposed A)
    etat_d = nc.dram_tensor("etat", [D, 2 * ROWS], bf16, kind="ExternalInput").ap()
    # Ap rows aligned with bx rows: row i = (i+1) * a_i
    ap_d = nc.dram_tensor("ap", [ROWS, D], bf16, kind="ExternalInput").ap()
    um_d = nc.dram_tensor("umask", [C, C], f32, kind="ExternalInput").ap()
    out_d = nc.dram_tensor("out", [ROWS, H], bf16, kind="ExternalOutput").ap()

    mult = mybir.AluOpType.mult

    with tile.TileContext(nc) as tc:
        with (
            tc.tile_pool(name="const", bufs=1) as cpool,
            tc.tile_pool(name="bxp", bufs=4) as bxpool,
            tc.tile_pool(name="outlo", bufs=4) as outlopool,
            tc.tile_pool(name="outhi", bufs=4) as outhipool,
            tc.tile_pool(name="eat", bufs=4) as eatpool,
            tc.tile_pool(name="app", bufs=4) as appool,
            tc.tile_pool(name="sm", bufs=4) as smpool,
            tc.tile_pool(name="mp", bufs=2) as mpool,
            tc.tile_pool(name="ps_s", bufs=2, space="PSUM") as ps_s,
            tc.tile_pool(name="ps_a", bufs=2, space="PSUM") as ps_a,
            tc.tile_pool(name="ps_b", bufs=2, space="PSUM") as ps_b,
            tc.tile_pool(name="ps_m", bufs=2, space="PSUM") as ps_m,
        ):
            umask_s = cpool.tile([C, C], f32)

            bx_t = [None] * NT
            eat_t = [None] * NT
            ap_t = [None] * NT

            def load_group(gr, skip_bx=False):
                """DMA one group of G chunks (bx / etat / ap).

                etat lands duplicated on both partition halves: the lower
                copy feeds the score block and the state-apply row pair's
                lower stationary, the upper copy its upper stationary.
                Both copies come straight from HBM — an SBUF->SBUF hop
                serializes on a single DMA queue and arrives too late.
                Group 0 skips chunk 0, which the prologue fast-path loads
                separately so the pipeline starts without a full group."""
                t0 = gr * G
                sk = 2 if gr == 0 else 0
                n = G - sk
                EAT4 = eatpool.tile(
                    [2 * D, G * 2 * C], bf16, name=f"EAT4_{gr}", tag="EAT4"
                )
                src = etat_d[:, (t0 + sk) * 2 * C:(t0 + G) * 2 * C]
                nc.sync.dma_start(out=EAT4[0:D, sk * 2 * C:], in_=src)
                nc.sync.dma_start(out=EAT4[D:2 * D, sk * 2 * C:], in_=src)
                AP4 = appool.tile([C, G * D], bf16, name=f"AP4_{gr}", tag="AP4")
                nc.sync.dma_start(
                    out=AP4[:, sk * D:].rearrange("p (g d) -> p g d", g=n),
                    in_=ap_d[(t0 + sk) * C:(t0 + G) * C, :].rearrange(
                        "(g p) d -> p g d", g=n
                    ),
                )
                BX4 = bxpool.tile([C, G * H], bf16, name=f"BX4_{gr}", tag="BX4")
                if not skip_bx:
                    nc.sync.dma_start(
                        out=BX4[:, sk * H:].rearrange("p (g h) -> p g h", g=n),
                        in_=bx_d[(t0 + sk) * C:(t0 + G) * C, :].rearrange(
                            "(g p) h -> p g h", g=n
                        ),
                    )
                for q in range(sk, G):
                    t = t0 + q
                    bx_t[t] = BX4[:, q * H:(q + 1) * H]
                    eat_t[t] = EAT4
                    ap_t[t] = AP4[:, q * D:(q + 1) * D]
                return BX4

            def s_block(t):
                """Score block S'(t) on PE + mask on DVE (pipelined ahead)."""
                q = t % G
                Atp = eat_t[t][0:D, q * 2 * C + C:(q + 1) * 2 * C]
                Etp = eat_t[t][0:D, q * 2 * C:q * 2 * C + C]
                s_p = ps_s.tile([C, C], f32, name=f"s_p_{t}", tag="s_p")
                nc.tensor.matmul(
                    out=s_p[:], lhsT=Atp, rhs=Etp, start=True, stop=True,
                    skip_group_check=True,
                )
                St = smpool.tile([C, C], bf16, name=f"St_{t}", tag="St")
                nc.vector.tensor_tensor(
                    out=St[:], in0=s_p[:], in1=umask_s[:], op=mult,
                )
                return St

            # fast-path chunks 0-1: dedicated loads so the pipeline starts
            # without waiting on a full group. A single DMA queue completes
            # transfers sequentially (~0.7-0.9 us apiece early on), so the
            # first two groups' bx chunks are spread per-chunk across the
            # Sync / Act / GpSimd queues, each landing just before the
            # pipeline reaches it. eat0 holds both chunks 0-1 and both
            # partition halves so chunk 1's state apply needs no group-0
            # table; sync carries it first, then bx0 and the group tables.
            eat0 = cpool.tile([2 * D, 4 * C], bf16)
            nc.sync.dma_start(out=eat0[0:D, :], in_=etat_d[:, 0:4 * C])
            nc.sync.dma_start(out=eat0[D:2 * D, :], in_=etat_d[:, 0:4 * C])
            nc.sync.dma_start(out=umask_s[:], in_=um_d[:, :])
            bx0 = cpool.tile([C, H], bf16)
            nc.sync.dma_start(out=bx0[:], in_=bx_d[0:C, :])
            ap0 = cpool.tile([C, 2 * D], bf16)
            nc.gpsimd.dma_start(
                out=ap0[:].rearrange("p (g d) -> p g d", g=2),
                in_=ap_d[0:2 * C, :].rearrange("(g p) d -> p g d", g=2),
            )
            bx1 = cpool.tile([C, H], bf16)
            nc.scalar.dma_start(out=bx1[:], in_=bx_d[C:2 * C, :])
            for t in (0, 1):
                bx_t[t] = (bx0 if t == 0 else bx1)[:, :]
                eat_t[t] = eat0
                ap_t[t] = ap0[:, t * D:(t + 1) * D]

            St_next = s_block(0)
            BXg0 = load_group(0, skip_bx=True)
            BXg1 = load_group(1, skip_bx=True)
            # group-0 chunks 2-3 and group-1 chunks per-pair across queues
            nc.scalar.dma_start(
                out=BXg0[:, 2 * H:3 * H], in_=bx_d[2 * C:3 * C, :],
            )
            nc.gpsimd.dma_start(
                out=BXg0[:, 3 * H:4 * H], in_=bx_d[3 * C:4 * C, :],
            )
            nc.scalar.dma_start(
                out=BXg1[:, 0:2 * H].rearrange("p (g h) -> p g h", g=2),
                in_=bx_d[G * C:(G + 2) * C, :].rearrange("(g p) h -> p g h", g=2),
            )
            nc.gpsimd.dma_start(
                out=BXg1[:, 2 * H:4 * H].rearrange("p (g h) -> p g h", g=2),
                in_=bx_d[(G + 2) * C:(G + 4) * C, :].rearrange(
                    "(g p) h -> p g h", g=2
                ),
            )

            M_p = None
            M_s = None
            for t in range(NT):
                b, c = divmod(t, NCH)
                q = t % G
                BX = bx_t[t]
                Etp = eat_t[t][0:D, q * 2 * C:q * 2 * C + C]
                EtpD = eat_t[t][D:2 * D, q * 2 * C:q * 2 * C + C]

                if q == 0 and t // G + 2 < NT // G:
                    load_group(t // G + 2)

                if c == 0:
                    # exactly-folded rank-D state: partitions 0:64 hold
                    # M[:, 0:384], partitions 64:128 hold M[:, 384:768]
                    M_p = ps_m.tile([2 * D, HF], f32, name=f"M_p_b{b}", tag="M_p")

                # M += y^T-outer-a as a col-group pair: both halves share
                # the ap stationary and write opposite output halves, so
                # they co-execute (384-col wall). skip_group_check: the
                # sim's group guard can't express this read-between-
                # accumulations pattern; the pending-zero accumulate
                # semantics and Tile's HW sync are unaffected.
                if c < NCH - 1:
                    nc.tensor.matmul(
                        out=M_p[0:D, :],
                        lhsT=ap_t[t],
                        rhs=BX[:, 0:HF],
                        start=(c == 0),
                        stop=True,
                        skip_group_check=True,
                    )
                    nc.tensor.matmul(
                        out=M_p[D:2 * D, :],
                        lhsT=ap_t[t],
                        rhs=BX[:, HF:H],
                        start=(c == 0),
                        stop=True,
                        skip_group_check=True,
                    )

                St = St_next
                if t + 1 < NT:
                    St_next = s_block(t + 1)

                # acc = St^T @ BX (+ Et'^T @ M); two [C, 384] PSUM tiles in
                # separate banks so each matmul stays bank-local
                out_a = ps_a.tile([C, HF], f32, name=f"out_a_{t}", tag="out_a")
                out_b = ps_b.tile([C, HF], f32, name=f"out_b_{t}", tag="out_b")
                nc.tensor.matmul(
                    out=out_a[:], lhsT=St[:], rhs=BX[:, 0:HF],
                    start=True, stop=(c == 0), skip_group_check=True,
                )
                nc.tensor.matmul(
                    out=out_b[:], lhsT=St[:], rhs=BX[:, HF:H],
                    start=True, stop=(c == 0), skip_group_check=True,
                )

                if c > 0:
                    # state apply as a row-group pair: 64-contract halves on
                    # opposite input partition halves co-execute
                    nc.tensor.matmul(
                        out=out_a[:], lhsT=Etp, rhs=M_s[0:D, :],
                        start=False, stop=True, skip_group_check=True,
                    )
                    nc.tensor.matmul(
                        out=out_b[:], lhsT=EtpD, rhs=M_s[D:2 * D, :],
                        start=False, stop=True, skip_group_check=True,
                    )

                # snapshot M for the NEXT chunk (reads M_p after this chunk's
                # update, before the next one; the Act engine runs it as soon
                # as the update's semaphore fires). This chain — update,
                # snapshot, next update — is the fast-clock critical path.
                if t + 1 < NT and (t + 1) % NCH != 0:
                    M_s = mpool.tile([2 * D, HF], bf16, name=f"M_s_{t + 1}", tag="M_s")
                    nc.scalar.copy(out=M_s[:], in_=M_p[:])

                # acc -> bf16, DVE for cols 0:384 into its own tile, Act for
                # cols 384:768 into another — no shared-tile serialization
                if q == 0:
                    OUTLO = outlopool.tile(
                        [C, G * HF], bf16, name=f"OUTLO_{t}", tag="OUTLO"
                    )
                    OUTHI = outhipool.tile(
                        [C, G * HF], bf16, name=f"OUTHI_{t}", tag="OUTHI"
                    )
                nc.vector.tensor_scalar_add(
                    out=OUTLO[:, q * HF:(q + 1) * HF], in0=out_a[:], scalar1=0.0,
                )
                nc.scalar.copy(
                    out=OUTHI[:, q * HF:(q + 1) * HF], in_=out_b[:],
                )
                if t >= NT - G:
                    # final group: store per chunk on both queues so the
                    # tail drain is small concurrent transfers
                    nc.sync.dma_start(
                        out=out_d[t * C:(t + 1) * C, 0:HF],
                        in_=OUTLO[:, q * HF:(q + 1) * HF],
                    )
                    nc.gpsimd.dma_start(
                        out=out_d[t * C:(t + 1) * C, HF:H],
                        in_=OUTHI[:, q * HF:(q + 1) * HF],
                    )
                elif q == G - 1:
                    t0 = t - G + 1
                    nc.sync.dma_start(
                        out=out_d[t0 * C:(t + 1) * C, 0:HF].rearrange(
                            "(g p) h -> p g h", g=G
                        ),
                        in_=OUTLO[:].rearrange("p (g h) -> p g h", g=G),
                    )
                    nc.gpsimd.dma_start(
                        out=out_d[t0 * C:(t + 1) * C, HF:H].rearrange(
                            "(g p) h -> p g h", g=G
                        ),
                        in_=OUTHI[:].rearrange("p (g h) -> p g h", g=G),
                    )

    # Adjacent PE matmuls often share a stationary operand; legalization has
    # already paired each matmul with a standalone InstLdweights, so drop the
    # redundant reloads. The key includes the PE array tile position: the
    # same weights loaded into a different array quadrant is a genuine
    # reload (and what lets paired matmuls co-execute).
    ndropped = 0
    for blk in nc.m.functions[0].blocks:
        keep = []
        last_w = None
        for inst in blk.instructions:
            if getattr(inst, "engine", None) == mybir.EngineType.PE:
                if isinstance(inst, mybir.InstLdweights):
                    w = inst.ins[0]
                    wkey = (
                        w.memref,
                        w.offset,
                        str(w.ap),
                        str(getattr(inst, "tile_position", None)),
                        str(getattr(inst, "tile_size", None)),
                    )
                    if (
                        last_w is not None
                        and wkey == last_w
                        and not inst.has_wait()
                    ):
                        ndropped += 1
                        continue
                    last_w = wkey
                elif not isinstance(inst, mybir.InstMatmult):
                    last_w = None
            keep.append(inst)
        blk.instructions = keep
    if os.environ.get("BASS_DEBUG_FUSE"):
        print(f"[kernel] redundant ldweights dropped: {ndropped}", file=sys.stderr)

    nc.compile()
    _compiled[key] = nc
    return nc


def _np_umask():
    i = np.arange(C)
    return (i[:, None] < i[None, :]).astype(np.float32)


def _in_maps(bert_x, x, ae, w):
    import ml_dtypes

    bf16 = ml_dtypes.bfloat16
    bert_x = np.asarray(bert_x, dtype=np.float32)
    x = np.asarray(x)
    ae = np.asarray(ae, dtype=np.float32)
    w = np.asarray(w, dtype=np.float32)

    E = ae[x.reshape(-1)]                     # [B*L, D]
    A = E @ w                                 # [B*L, D]
    jp1 = (np.arange(L, dtype=np.float64) + 1.0).astype(np.float32)
    Ap = (A.reshape(B, L, D) * jp1[None, :, None]).reshape(B * L, D)
    Einv = (E.reshape(B, L, D) / jp1[None, :, None]).reshape(B * L, D)

    bx16 = np.ascontiguousarray(bert_x.reshape(B * L, H).astype(bf16))
    ap16 = np.ascontiguousarray(Ap.astype(bf16))

    # etat per core: [D, 2*ROWS]; per global chunk g: [Et'_g | At'_g]
    Ech = Einv.reshape(B, NCH, C, D).astype(bf16)
    Ach = Ap.reshape(B, NCH, C, D).astype(bf16)
    pair = np.stack([Ech, Ach], axis=2)       # [B, NCH, 2, C, D]
    pair = pair.transpose(0, 4, 1, 2, 3)      # [B, D, NCH, 2, C]

    umask = _np_umask()
    maps = []
    for k in range(NCORES):
        et = np.ascontiguousarray(
            pair[k * BPC:(k + 1) * BPC].transpose(1, 0, 2, 3, 4).reshape(D, 2 * ROWS)
        )
        maps.append(
            {
                "bx": bx16[k * BPC * L:(k + 1) * BPC * L],
                "etat": et,
                "ap": ap16[k * BPC * L:(k + 1) * BPC * L],
                "umask": umask,
            }
        )
    return maps


def _run(bert_x, x, ae, w, trace=False):
    from concourse import bass_utils

    nc = _build()
    maps = _in_maps(bert_x, x, ae, w)
    res = bass_utils.run_bass_kernel_spmd(
        nc, maps, core_ids=list(range(NCORES)), trace=trace
    )
    acc = np.concatenate(
        [
            res.results[k]["out"].astype(np.float32).reshape(BPC, L, H)
            for k in range(NCORES)
        ],
        axis=0,
    )
    out = np.asarray(bert_x, dtype=np.float32) + acc
    return out, res


def kernel(bert_x, x, ae, w):
    out, _ = _run(bert_x, x, ae, w, trace=False)
    return out
